# revision 1
# baseline (speedup 1.0000x reference)
"""Distributed gaussian-mask attention for trn2 (8 NeuronCores, SPMD).

Problem: B=2, S=2048, H=1024, 16 heads, hd=64.
  q/k/v = x@W*, dif = q - k, score = exp(-0.5 * dif @ dif^T),
  prob = score * triu(ones,k=1), ctx = prob @ v, out = ctx @ Wo + bo.
  (bq/bk/bv are zeros by construction -- folded out; dif = x @ (Wq-Wk).)

Sharding (uniform SPMD program, data-only per-core differences):
  - Head parallel: core c owns heads (2c, 2c+1) = 128 feature columns of
    Wq/Wk/Wv.  Each core computes D^T = (Wq-Wk)c^T-proj and V for ALL
    tokens of its 2 heads, runs the full (anti-)causal attention
    triangle locally (no collective), producing ctx^T [128, 4096].
  - One AllToAll per batch (cc_dim=Free) re-shards ctx from head-major
    to token-major: core c ends with full-H ctx^T for tokens
    [b, 256c:256c+256) of each batch, then does its 1/8 of the output
    projection with the full Wo.

Precision: score path runs x(fp16) @ Wd(fp16) -> D^T(f32r),
  D^T @ D^T (f32r, K=64 full-rate) -> exp (fp32 ACT) -> A (bf16);
  value path V/ctx/out-proj in bf16.  PSUM accumulation fp32.
"""
import numpy as np
import ml_dtypes

import concourse.bass as bass
import concourse.bacc as bacc
import concourse.mybir as mybir
import concourse.tile as tile
from concourse.bass_utils import run_bass_kernel_spmd

FP = mybir.dt.float32
F16 = mybir.dt.float16
BF = mybir.dt.bfloat16
F32R = mybir.dt.float32r
AF = mybir.ActivationFunctionType

NC = 8
B, S, H, NH, HD = 2, 2048, 1024, 16, 64
T = B * S            # 4096 tokens
QB = 256             # query block
KB = 128             # key block
NQB = S // QB        # 8 query blocks per batch
NKB = S // KB        # 16 key blocks per batch

_cached = {}
INTERLEAVE_OUTPROJ = False


def _build(dbg=False):
    nc = bacc.Bacc("TRN2", target_bir_lowering=False, debug=False, num_devices=NC)

    xT = nc.dram_tensor("xT", [H, T], F16, kind="ExternalInput")
    Wqc = nc.dram_tensor("Wqc", [H, 128], FP, kind="ExternalInput")
    Wkc = nc.dram_tensor("Wkc", [H, 128], FP, kind="ExternalInput")
    Wvc = nc.dram_tensor("Wvc", [H, 128], F16, kind="ExternalInput")
    Wob = nc.dram_tensor("Wob", [H, H], BF, kind="ExternalInput")
    bo_d = nc.dram_tensor("bo", [H], FP, kind="ExternalInput")
    mask_d = nc.dram_tensor("maskbf", [128, 128], BF, kind="ExternalInput")
    out_d = nc.dram_tensor("out", [H, 2 * QB], FP, kind="ExternalOutput")
    if dbg:
        dbg_dT = nc.dram_tensor("dbg_dT", [8 * 128, 512], F32R, kind="ExternalOutput")
        dbg_V = nc.dram_tensor("dbg_V", [8 * 128, 512], BF, kind="ExternalOutput")
        dbg_ctxT = nc.dram_tensor(
            "dbg_ctxT", [B * 128, 2048], BF, kind="ExternalOutput"
        )
        dbg_ctxg = nc.dram_tensor(
            "dbg_ctxg", [B * 8 * 128, 256], BF, kind="ExternalOutput"
        )

    with tile.TileContext(nc) as tc:
        with (
            tc.tile_pool(name="res", bufs=1) as res,      # resident SBUF
            tc.tile_pool(name="stream", bufs=3) as strm,  # streamed SBUF
            tc.tile_pool(name="dram", bufs=1, space="DRAM") as dram,
        ):
            # ---------------- constants / weights in ----------------
            mask_t = res.tile([128, 128], BF, tag="mask")
            nc.sync.dma_start(mask_t[:], mask_d[:])
            bo_t = res.tile([128, 8], FP, tag="bo")
            nc.sync.dma_start(bo_t[:], bo_d[:].rearrange("(f p) -> p f", p=128))

            wq = []
            wk = []
            wd = []
            wv = []
            wo = []
            for k in range(8):
                wq_k = strm.tile([128, 128], FP, tag="wqk", name=f"wq{k}")
                wk_k = strm.tile([128, 128], FP, tag="wkk", name=f"wk{k}")
                nc.sync.dma_start(wq_k[:], Wqc[k * 128:(k + 1) * 128, :])
                nc.sync.dma_start(wk_k[:], Wkc[k * 128:(k + 1) * 128, :])
                wd_k = res.tile([128, 128], F16, tag=f"wd{k}", name=f"wd{k}")
                nc.vector.tensor_sub(wd_k[:], wq_k[:], wk_k[:])
                wd.append(wd_k)
                wv_k = res.tile([128, 128], F16, tag=f"wv{k}", name=f"wv{k}")
                nc.sync.dma_start(wv_k[:], Wvc[k * 128:(k + 1) * 128, :])
                wv.append(wv_k)
                wo_k = res.tile([128, 1024], BF, tag=f"wo{k}", name=f"wo{k}")
                nc.sync.dma_start(wo_k[:], Wob[k * 128:(k + 1) * 128, :])
                wo.append(wo_k)

            # resident outputs of the projections
            dT = [res.tile([128, 512], F32R, tag=f"dT{i}", name=f"dT{i}")
                  for i in range(8)]                  # D^T  [128 feat, 4096 tok]
            Vg = [res.tile([128, 512], BF, tag=f"Vg{i}", name=f"Vg{i}")
                  for i in range(8)]                  # V    [tok, feat] 4 tiles/grp

            # ---------------- projections: D^T and V ----------------
            with tc.tile_pool(name="psp", bufs=1, space="PSUM") as psp:
                for half in range(2):                 # token halves (2048 each)
                    xk_tiles = []
                    for k in range(8):
                        xk = strm.tile([128, 2048], F16, tag="xk", name=f"xk{half}_{k}")
                        nc.sync.dma_start(
                            xk[:], xT[k * 128:(k + 1) * 128,
                                      half * 2048:(half + 1) * 2048]
                        )
                        xk_tiles.append(xk)
                    pd = [psp.tile([128, 512], FP, tag=f"pd{j}", name=f"pd{half}_{j}")
                          for j in range(4)]
                    pv = [psp.tile([128, 512], FP, tag=f"pv{j}", name=f"pv{half}_{j}")
                          for j in range(4)]
                    for k in range(8):
                        xk = xk_tiles[k]
                        for j in range(4):            # 512-token chunks -> D^T
                            nc.tensor.matmul(
                                pd[j][:], wd[k][:], xk[:, j * 512:(j + 1) * 512],
                                start=(k == 0), stop=(k == 7),
                            )
                        for t in range(16):           # 128-token tiles -> V
                            nc.tensor.matmul(
                                pv[t // 4][:, (t % 4) * 128:(t % 4 + 1) * 128],
                                xk[:, t * 128:(t + 1) * 128], wv[k][:],
                                start=(k == 0 and t % 4 == 0), stop=(k == 7),
                                skip_group_check=True,
                            )
                    for j in range(4):
                        nc.vector.tensor_copy(dT[half * 4 + j][:], pd[j][:])
                        nc.vector.tensor_copy(Vg[half * 4 + j][:], pv[j][:])
                if dbg:
                    for i in range(8):
                        nc.sync.dma_start(
                            dbg_dT[i * 128:(i + 1) * 128, :], dT[i][:]
                        )
                        nc.sync.dma_start(
                            dbg_V[i * 128:(i + 1) * 128, :], Vg[i][:]
                        )

            # ---------------- attention (local, 2 heads) ----------------
            # ctx^T accumulates into per-batch SBUF tiles [128, 2048] bf16
            ctxT = [res.tile([128, 2048], BF, tag=f"ctxT{b}", name=f"ctxT{b}")
                    for b in range(B)]

            ctxg = [[], []]                           # [128, 256] bf16 x 8 per b

            def emit_outproj_group(b, fo):
                po = _pso[0].tile([128, 256], FP, tag="po", name=f"po{b}_{fo}")
                for k in range(8):
                    nc.tensor.matmul(
                        po[:], wo[k][:, fo * 128:(fo + 1) * 128],
                        ctxg[b][k][:],
                        start=(k == 0), stop=(k == 7),
                    )
                ot = strm.tile([128, 256], FP, tag="ot", bufs=4,
                               name=f"ot{b}_{fo}")
                nc.scalar.activation(
                    ot[:], po[:], AF.Identity,
                    bias=bo_t[:, fo:fo + 1], scale=1.0,
                )
                nc.sync.dma_start(
                    out_d[fo * 128:(fo + 1) * 128, b * 256:(b + 1) * 256], ot[:]
                )

            with (
                tc.tile_pool(name="pssc", bufs=4, space="PSUM") as pssc,
                tc.tile_pool(name="pscx", bufs=2, space="PSUM") as pscx,
                tc.tile_pool(name="pso", bufs=2, space="PSUM") as pso,
            ):
                _pso = [pso]
                for b in range(B):
                    for qb in range(NQB):
                        toff = b * S + qb * QB        # global query token offset
                        qt, qc = toff // 512, toff % 512
                        pc = pscx.tile([128, QB], FP, tag="pc", name=f"pc{b}_{qb}")
                        for kb in range(2 * qb, NKB):
                            koff = b * S + kb * KB
                            kt, kc = koff // 512, koff % 512
                            n = 128 if kb == 2 * qb else QB
                            for h in range(2):
                                ps = pssc.tile([128, QB], FP, tag="ps",
                                               name=f"ps{b}_{qb}_{kb}_{h}")
                                nc.tensor.matmul(
                                    ps[:, 0:n],
                                    dT[kt][h * 64:(h + 1) * 64, kc:kc + 128],
                                    dT[qt][h * 64:(h + 1) * 64, qc:qc + n],
                                    start=True, stop=True,
                                )
                                at = strm.tile([128, QB], BF, tag="at", bufs=6,
                                               name=f"at{b}_{qb}_{kb}_{h}")
                                nc.scalar.activation(
                                    at[:, 0:n], ps[:, 0:n], AF.Exp, scale=-0.5
                                )
                                if kb == 2 * qb:
                                    nc.vector.tensor_mul(
                                        at[:, 0:128], at[:, 0:128], mask_t[:]
                                    )
                                elif kb == 2 * qb + 1:
                                    nc.vector.tensor_mul(
                                        at[:, 128:256], at[:, 128:256], mask_t[:]
                                    )
                                nc.tensor.matmul(
                                    pc[h * 64:(h + 1) * 64, 0:n],
                                    Vg[(b * 16 + kb) // 4][
                                        :, ((b * 16 + kb) % 4) * 128 + h * 64:
                                           ((b * 16 + kb) % 4) * 128 + h * 64 + 64],
                                    at[:, 0:n],
                                    start=(kb == 2 * qb),
                                    stop=(kb == NKB - 1),
                                    skip_group_check=True,
                                )
                        nc.vector.tensor_copy(
                            ctxT[b][:, qb * QB:(qb + 1) * QB], pc[:]
                        )
                        if b == 1 and INTERLEAVE_OUTPROJ:
                            # interleave batch-0 output projection into the
                            # batch-1 attention stream (A2A0 already landed)
                            emit_outproj_group(0, qb)

                    # AllToAll for this batch (head-shard -> token-shard);
                    # overlaps the next batch's attention on PE/ACT.
                    # Partition-split semantics: slab j of the input goes to
                    # rank j; out slab i arrives from rank i.
                    cc_in = dram.tile([1024, 256], BF, name=f"cc_in{b}")
                    cc_out = dram.tile([1024, 256], BF, name=f"cc_out{b}")
                    for j in range(8):
                        nc.sync.dma_start(
                            cc_in[j * 128:(j + 1) * 128, :],
                            ctxT[b][:, j * 256:(j + 1) * 256],
                        )
                    nc.gpsimd.collective_compute(
                        "AllToAll",
                        mybir.AluOpType.bypass,
                        replica_groups=[list(range(NC))],
                        ins=[cc_in[:].opt()],
                        outs=[cc_out[:].opt()],
                    )
                    for k in range(8):
                        g = res.tile([128, 256], BF, tag=f"cg{b}_{k}",
                                     name=f"cg{b}_{k}")
                        nc.sync.dma_start(g[:], cc_out[k * 128:(k + 1) * 128, :])
                        ctxg[b].append(g)
                    if dbg:
                        nc.sync.dma_start(
                            dbg_ctxT[b * 128:(b + 1) * 128, :], ctxT[b][:]
                        )
                        for k in range(8):
                            nc.sync.dma_start(
                                dbg_ctxg[(b * 8 + k) * 128:(b * 8 + k + 1) * 128, :],
                                ctxg[b][k][:],
                            )

                # batch-1 output projection (batch 0 interleaved above when
                # INTERLEAVE_OUTPROJ is set)
                if not INTERLEAVE_OUTPROJ:
                    for fo in range(8):
                        emit_outproj_group(0, fo)
                for fo in range(8):
                    emit_outproj_group(1, fo)

    nc.compile()
    return nc


def kernel(**inputs):
    x = np.asarray(inputs["x"], np.float32)
    Wq = np.asarray(inputs["Wq"], np.float32)
    Wk = np.asarray(inputs["Wk"], np.float32)
    Wv = np.asarray(inputs["Wv"], np.float32)
    Wo = np.asarray(inputs["Wo"], np.float32)
    bo = np.asarray(inputs["bo"], np.float32)
    # bq/bk/bv are zeros by the problem's input spec; dif = x @ (Wq - Wk)
    # and v = x @ Wv absorb them exactly when zero.

    if "nc" not in _cached:
        _cached["nc"] = _build()
    nc = _cached["nc"]

    xT = np.ascontiguousarray(x.reshape(T, H).T).astype(np.float16)
    Wob = Wo.astype(ml_dtypes.bfloat16)
    maskbf = np.tril(np.ones((128, 128), np.float32), -1).astype(ml_dtypes.bfloat16)

    in_maps = []
    for c in range(NC):
        cols = slice(c * 128, (c + 1) * 128)
        in_maps.append({
            "xT": xT,
            "Wqc": np.ascontiguousarray(Wq[:, cols]),
            "Wkc": np.ascontiguousarray(Wk[:, cols]),
            "Wvc": np.ascontiguousarray(Wv[:, cols]).astype(np.float16),
            "Wob": Wob,
            "bo": bo,
            "maskbf": maskbf,
        })

    res = run_bass_kernel_spmd(nc, in_maps, core_ids=list(range(NC)))

    out = np.empty((B, S, H), np.float32)
    for c in range(NC):
        oT = res.results[c]["out"]                    # [H, 512]
        for b in range(B):
            out[b, c * QB:(c + 1) * QB, :] = oT[:, b * QB:(b + 1) * QB].T
    return out



# revision 2
# speedup vs baseline: 1.5164x; 1.5164x over previous
"""Distributed gaussian-mask attention for trn2 (8 NeuronCores, SPMD).

Problem: B=2, S=2048, H=1024, 16 heads, hd=64.
  q/k/v = x@W*, dif = q - k, score = exp(-0.5 * dif @ dif^T),
  prob = score * triu(ones,k=1), ctx = prob @ v, out = ctx @ Wo + bo.
  (bq/bk/bv are zeros by construction -- folded out; dif = x @ (Wq-Wk).)

Sharding (uniform SPMD program, data-only per-core differences):
  - Head parallel: core c owns heads (2c, 2c+1) = 128 feature columns of
    Wq/Wk/Wv.  Each core computes D^T = x @ (Wq-Wk)c and V for ALL
    tokens of its 2 heads, runs the full anti-causal attention triangle
    locally (no collective), producing ctx^T [128, 4096].
  - One AllToAll per batch (partition-split) re-shards ctx from
    head-major to token-major: core c ends with full-H ctx^T for tokens
    [b, 256c:256c+256) of each batch, then does its 1/8 of the output
    projection with the full Wo.

Pipeline structure (v2): the attention inner loop is software-pipelined
so the PE never head-of-line blocks on ACT:
  PE:  score(kb,h0) score(kb,h1) ctx(kb-1,h0) ctx(kb-1,h1) score(kb+1)...
  ACT: exp(kb-1)    exp(kb)      ...   (one [128,1024] instr per kb,
                                        covering both heads)
Scores for both heads of one key block land in one 2-bank PSUM tile
(bank0 = h0, bank1 = h1, 512 query cols each); h0/h1 matmuls pack into
disjoint PE row groups (K=64) resp. col groups (ctx, M=64) and overlap.
Diagonal key blocks use ragged n (per-element has_written handles the
ragged accumulation) + a [128,128] tril mask multiply on DVE.

Precision: x/Wd/dT fp16, V/ctx/at/out-proj bf16, PSUM fp32.
"""
import numpy as np
import ml_dtypes

import concourse.bass as bass
import concourse.bacc as bacc
import concourse.mybir as mybir
import concourse.tile as tile
from concourse.bass_utils import run_bass_kernel_spmd

FP = mybir.dt.float32
F16 = mybir.dt.float16
BF = mybir.dt.bfloat16
AF = mybir.ActivationFunctionType

NC = 8
B, S, H, NH, HD = 2, 2048, 1024, 16, 64
T = B * S            # 4096 tokens
QB = 512             # query block
KB = 128             # key block
NQB = S // QB        # 4 query blocks per batch
NKB = S // KB        # 16 key blocks per batch

_cached = {}


def _build():
    nc = bacc.Bacc("TRN2", target_bir_lowering=False, debug=False, num_devices=NC)

    xT = nc.dram_tensor("xT", [H, T], F16, kind="ExternalInput")
    Wqc = nc.dram_tensor("Wqc", [H, 128], FP, kind="ExternalInput")
    Wkc = nc.dram_tensor("Wkc", [H, 128], FP, kind="ExternalInput")
    Wvc = nc.dram_tensor("Wvc", [H, 128], F16, kind="ExternalInput")
    Wob = nc.dram_tensor("Wob", [H, H], BF, kind="ExternalInput")
    bo_d = nc.dram_tensor("bo", [H], FP, kind="ExternalInput")
    mask_d = nc.dram_tensor("maskbf", [128, 128], BF, kind="ExternalInput")
    out_d = nc.dram_tensor("out", [H, 512], FP, kind="ExternalOutput")

    with tile.TileContext(nc) as tc:
        with (
            tc.tile_pool(name="res", bufs=1) as res,      # resident SBUF
            tc.tile_pool(name="stream", bufs=3) as strm,  # streamed SBUF
            tc.tile_pool(name="dram", bufs=1, space="DRAM") as dram,
        ):
            # ---------------- constants / weights in ----------------
            mask_t = res.tile([128, 128], BF, tag="mask")
            nc.sync.dma_start(mask_t[:], mask_d[:])
            bo_t = res.tile([128, 8], FP, tag="bo")
            nc.sync.dma_start(bo_t[:], bo_d[:].rearrange("(f p) -> p f", p=128))

            wd = []
            wv = []
            wo = []
            for k in range(8):
                wq_k = strm.tile([128, 128], FP, tag="wqk", name=f"wq{k}")
                wk_k = strm.tile([128, 128], FP, tag="wkk", name=f"wk{k}")
                nc.sync.dma_start(wq_k[:], Wqc[k * 128:(k + 1) * 128, :])
                nc.sync.dma_start(wk_k[:], Wkc[k * 128:(k + 1) * 128, :])
                wd_k = res.tile([128, 128], F16, tag=f"wd{k}", name=f"wd{k}")
                nc.vector.tensor_sub(wd_k[:], wq_k[:], wk_k[:])
                wd.append(wd_k)
                wv_k = res.tile([128, 128], F16, tag=f"wv{k}", name=f"wv{k}")
                nc.sync.dma_start(wv_k[:], Wvc[k * 128:(k + 1) * 128, :])
                wv.append(wv_k)
                wo_k = res.tile([128, 1024], BF, tag=f"wo{k}", name=f"wo{k}")
                nc.sync.dma_start(wo_k[:], Wob[k * 128:(k + 1) * 128, :])
                wo.append(wo_k)

            # resident outputs of the projections
            dT = [res.tile([128, 512], F16, tag=f"dT{i}", name=f"dT{i}")
                  for i in range(8)]                  # D^T  [128 feat, 4096 tok]
            Vg = [res.tile([128, 512], BF, tag=f"Vg{i}", name=f"Vg{i}")
                  for i in range(8)]                  # V    [tok, feat] 4 tiles/grp

            # ---------------- projections: D^T and V ----------------
            with tc.tile_pool(name="psp", bufs=1, space="PSUM") as psp:
                for half in range(2):                 # token halves (2048 each)
                    xk_tiles = []
                    for k in range(8):
                        xk = strm.tile([128, 2048], F16, tag="xk", name=f"xk{half}_{k}")
                        nc.sync.dma_start(
                            xk[:], xT[k * 128:(k + 1) * 128,
                                      half * 2048:(half + 1) * 2048]
                        )
                        xk_tiles.append(xk)
                    pd = [psp.tile([128, 512], FP, tag=f"pd{j}", name=f"pd{half}_{j}")
                          for j in range(4)]
                    pv = [psp.tile([128, 512], FP, tag=f"pv{j}", name=f"pv{half}_{j}")
                          for j in range(4)]
                    for k in range(8):
                        xk = xk_tiles[k]
                        for j in range(4):            # 512-token chunks -> D^T
                            nc.tensor.matmul(
                                pd[j][:], wd[k][:], xk[:, j * 512:(j + 1) * 512],
                                start=(k == 0), stop=(k == 7),
                            )
                        for t in range(16):           # 128-token tiles -> V
                            nc.tensor.matmul(
                                pv[t // 4][:, (t % 4) * 128:(t % 4 + 1) * 128],
                                xk[:, t * 128:(t + 1) * 128], wv[k][:],
                                start=(k == 0 and t % 4 == 0), stop=(k == 7),
                                skip_group_check=True,
                            )
                    for j in range(4):
                        nc.vector.tensor_copy(dT[half * 4 + j][:], pd[j][:])
                        nc.vector.tensor_copy(Vg[half * 4 + j][:], pv[j][:])

            # ---------------- attention (local, 2 heads) ----------------
            # ctx^T accumulates into per-batch SBUF tiles [128, 2048] bf16
            ctxT = [res.tile([128, 2048], BF, tag=f"ctxT{b}", name=f"ctxT{b}")
                    for b in range(B)]

            ctxg = [[], []]                           # [128, 256] bf16 x 8 per b

            def emit_outproj_group(b, fo):
                po = _pso[0].tile([128, 512], FP, tag="po", name=f"po{b}_{fo}")
                for k in range(8):
                    nc.tensor.matmul(
                        po[:, 0:256], wo[k][:, fo * 128:(fo + 1) * 128],
                        ctxg[b][k][:],
                        start=(k == 0), stop=(k == 7),
                    )
                ot = strm.tile([128, 256], FP, tag="ot", bufs=4,
                               name=f"ot{b}_{fo}")
                nc.scalar.activation(
                    ot[:], po[:, 0:256], AF.Identity,
                    bias=bo_t[:, fo:fo + 1], scale=1.0,
                )
                nc.sync.dma_start(
                    out_d[fo * 128:(fo + 1) * 128, b * 256:(b + 1) * 256], ot[:]
                )

            with (
                tc.tile_pool(name="pssc", bufs=2, space="PSUM") as pssc,
                tc.tile_pool(name="pscx", bufs=2, space="PSUM") as pscx,
                tc.tile_pool(name="pso", bufs=2, space="PSUM") as pso,
            ):
                _pso = [pso]
                for b in range(B):
                    for qb in range(NQB):
                        qt = b * 4 + qb               # query dT tile (512 tok)
                        pc = pscx.tile([128, QB], FP, tag="pc", name=f"pc{b}_{qb}")
                        prev = None                   # (at, n, g, m, first, last)
                        for kb in range(4 * qb, NKB):
                            koff = b * S + kb * KB
                            kt, kc = koff // 512, koff % 512
                            n = min(QB, (kb + 1) * KB - qb * QB)
                            j = kb - 4 * qb           # diag sub-block index
                            # -- scores for both heads into one 2-bank tile
                            ps = pssc.tile([128, 1024], FP, tag="ps",
                                           name=f"ps{b}_{qb}_{kb}")
                            for h in range(2):
                                nc.tensor.matmul(
                                    ps[:, 512 * h:512 * h + n],
                                    dT[kt][h * 64:(h + 1) * 64, kc:kc + 128],
                                    dT[qt][h * 64:(h + 1) * 64, 0:n],
                                    start=True, stop=True,
                                )
                            # -- exp on ACT (wide instr; both heads when full)
                            at = strm.tile([128, 1024], BF, tag="at", bufs=4,
                                           name=f"at{b}_{qb}_{kb}")
                            if n == QB:
                                nc.scalar.activation(
                                    at[:], ps[:], AF.Exp, scale=-0.5
                                )
                            else:
                                for h in range(2):
                                    nc.scalar.activation(
                                        at[:, 512 * h:512 * h + n],
                                        ps[:, 512 * h:512 * h + n],
                                        AF.Exp, scale=-0.5,
                                    )
                            if j < 4:                 # diagonal: tril mask on DVE
                                for h in range(2):
                                    lo = 512 * h + 128 * j
                                    nc.vector.tensor_mul(
                                        at[:, lo:lo + 128],
                                        at[:, lo:lo + 128], mask_t[:]
                                    )
                            # -- ctx for the PREVIOUS kb (one-block lag)
                            if prev is not None:
                                _emit_ctx(nc, pc, Vg, prev)
                            prev = (at, n, (b * 16 + kb) // 4, (b * 16 + kb) % 4,
                                    kb == 4 * qb, kb == NKB - 1)
                        _emit_ctx(nc, pc, Vg, prev)
                        nc.vector.tensor_copy(
                            ctxT[b][:, qb * QB:(qb + 1) * QB], pc[:]
                        )

                    # AllToAll for this batch (head-shard -> token-shard);
                    # overlaps the next batch's attention on PE/ACT.
                    # Partition-split semantics: slab j of the input goes to
                    # rank j; out slab i arrives from rank i.
                    cc_in = dram.tile([1024, 256], BF, name=f"cc_in{b}")
                    cc_out = dram.tile([1024, 256], BF, name=f"cc_out{b}")
                    for j in range(8):
                        nc.sync.dma_start(
                            cc_in[j * 128:(j + 1) * 128, :],
                            ctxT[b][:, j * 256:(j + 1) * 256],
                        )
                    nc.gpsimd.collective_compute(
                        "AllToAll",
                        mybir.AluOpType.bypass,
                        replica_groups=[list(range(NC))],
                        ins=[cc_in[:].opt()],
                        outs=[cc_out[:].opt()],
                    )
                    if b == 1:
                        # batch-0 output projection overlaps A2A(b1) transfer
                        for fo in range(8):
                            emit_outproj_group(0, fo)
                    for k in range(8):
                        g = res.tile([128, 256], BF, tag=f"cg{b}_{k}",
                                     name=f"cg{b}_{k}")
                        nc.sync.dma_start(g[:], cc_out[k * 128:(k + 1) * 128, :])
                        ctxg[b].append(g)

                for fo in range(8):
                    emit_outproj_group(1, fo)

    nc.compile()
    return nc


def _emit_ctx(nc, pc, Vg, prev):
    at, n, g, m, first, last = prev
    for h in range(2):
        nc.tensor.matmul(
            pc[h * 64:(h + 1) * 64, 0:n],
            Vg[g][:, m * 128 + h * 64:m * 128 + h * 64 + 64],
            at[:, 512 * h:512 * h + n],
            start=first, stop=last,
            skip_group_check=True,
        )


def kernel(**inputs):
    x = np.asarray(inputs["x"], np.float32)
    Wq = np.asarray(inputs["Wq"], np.float32)
    Wk = np.asarray(inputs["Wk"], np.float32)
    Wv = np.asarray(inputs["Wv"], np.float32)
    Wo = np.asarray(inputs["Wo"], np.float32)
    bo = np.asarray(inputs["bo"], np.float32)
    # bq/bk/bv are zeros by the problem's input spec; dif = x @ (Wq - Wk)
    # and v = x @ Wv absorb them exactly when zero.

    if "nc" not in _cached:
        _cached["nc"] = _build()
    nc = _cached["nc"]

    xT = np.ascontiguousarray(x.reshape(T, H).T).astype(np.float16)
    Wob = Wo.astype(ml_dtypes.bfloat16)
    maskbf = np.tril(np.ones((128, 128), np.float32), -1).astype(ml_dtypes.bfloat16)

    in_maps = []
    for c in range(NC):
        cols = slice(c * 128, (c + 1) * 128)
        in_maps.append({
            "xT": xT,
            "Wqc": np.ascontiguousarray(Wq[:, cols]),
            "Wkc": np.ascontiguousarray(Wk[:, cols]),
            "Wvc": np.ascontiguousarray(Wv[:, cols]).astype(np.float16),
            "Wob": Wob,
            "bo": bo,
            "maskbf": maskbf,
        })

    res = run_bass_kernel_spmd(nc, in_maps, core_ids=list(range(NC)))

    out = np.empty((B, S, H), np.float32)
    for c in range(NC):
        oT = res.results[c]["out"]                    # [H, 512]
        for b in range(B):
            out[b, c * 256:(c + 1) * 256, :] = oT[:, b * 256:(b + 1) * 256].T
    return out


# revision 6
# speedup vs baseline: 1.6034x; 1.0574x over previous
"""Distributed gaussian-mask attention for trn2 (8 NeuronCores, SPMD).

Problem: B=2, S=2048, H=1024, 16 heads, hd=64.
  q/k/v = x@W*, dif = q - k, score = exp(-0.5 * dif @ dif^T),
  prob = score * triu(ones,k=1), ctx = prob @ v, out = ctx @ Wo + bo.
  (bq/bk/bv are zeros by construction -- folded out; dif = x @ (Wq-Wk).)

Sharding (uniform SPMD program, data-only per-core differences):
  - Head parallel: core c owns heads (2c, 2c+1) = 128 feature columns of
    Wq/Wk/Wv.  Each core computes D^T = x @ (Wq-Wk)c and V for ALL
    tokens of its 2 heads, runs the full anti-causal attention triangle
    locally (no collective), producing ctx^T [128, 4096].
  - One AllToAll per batch (partition-split) re-shards ctx from
    head-major to token-major: core c ends with full-H ctx^T for tokens
    [b, 256c:256c+256) of each batch, then does its 1/8 of the output
    projection with the full Wo.

Pipeline structure (v2): the attention inner loop is software-pipelined
so the PE never head-of-line blocks on ACT:
  PE:  score(kb,h0) score(kb,h1) ctx(kb-1,h0) ctx(kb-1,h1) score(kb+1)...
  ACT: exp(kb-1)    exp(kb)      ...   (one [128,1024] instr per kb,
                                        covering both heads)
Scores for both heads of one key block land in one 2-bank PSUM tile
(bank0 = h0, bank1 = h1, 512 query cols each); h0/h1 matmuls pack into
disjoint PE row groups (K=64) resp. col groups (ctx, M=64) and overlap.
Diagonal key blocks use ragged n (per-element has_written handles the
ragged accumulation) + a [128,128] tril mask multiply on DVE.

Precision: x/Wd/dT fp16, V/ctx/at/out-proj bf16, PSUM fp32.
"""
import numpy as np
import ml_dtypes

import concourse.bass as bass
import concourse.bacc as bacc
import concourse.mybir as mybir
import concourse.tile as tile
from concourse.bass_utils import run_bass_kernel_spmd

FP = mybir.dt.float32
F16 = mybir.dt.float16
BF = mybir.dt.bfloat16
AF = mybir.ActivationFunctionType

NC = 8
B, S, H, NH, HD = 2, 2048, 1024, 16, 64
T = B * S            # 4096 tokens
QB = 512             # query block
KB = 128             # key block
NQB = S // QB        # 4 query blocks per batch
NKB = S // KB        # 16 key blocks per batch

_cached = {}


def _build():
    nc = bacc.Bacc("TRN2", target_bir_lowering=False, debug=False, num_devices=NC)

    xT = nc.dram_tensor("xT", [H, T], F16, kind="ExternalInput")
    Wqc = nc.dram_tensor("Wqc", [H, 128], FP, kind="ExternalInput")
    Wkc = nc.dram_tensor("Wkc", [H, 128], FP, kind="ExternalInput")
    Wvc = nc.dram_tensor("Wvc", [H, 128], F16, kind="ExternalInput")
    Wob = nc.dram_tensor("Wob", [H, H], BF, kind="ExternalInput")
    bo_d = nc.dram_tensor("bo", [H], FP, kind="ExternalInput")
    mask_d = nc.dram_tensor("maskbf", [128, 128], BF, kind="ExternalInput")
    out_d = nc.dram_tensor("out", [H, 512], FP, kind="ExternalOutput")

    with tile.TileContext(nc) as tc:
        with (
            tc.tile_pool(name="res", bufs=1) as res,      # resident SBUF
            tc.tile_pool(name="stream", bufs=3) as strm,  # streamed SBUF
            tc.tile_pool(name="dram", bufs=1, space="DRAM") as dram,
        ):
            # ---------------- constants / weights in ----------------
            mask_t = res.tile([128, 128], BF, tag="mask")
            nc.sync.dma_start(mask_t[:], mask_d[:])
            bo_t = res.tile([128, 8], FP, tag="bo")
            nc.sync.dma_start(bo_t[:], bo_d[:].rearrange("(f p) -> p f", p=128))

            wd = []
            wv = []
            wo = []
            for k in range(8):
                wq_k = strm.tile([128, 128], FP, tag="wqk", name=f"wq{k}")
                wk_k = strm.tile([128, 128], FP, tag="wkk", name=f"wk{k}")
                nc.sync.dma_start(wq_k[:], Wqc[k * 128:(k + 1) * 128, :])
                nc.sync.dma_start(wk_k[:], Wkc[k * 128:(k + 1) * 128, :])
                wd_k = res.tile([128, 128], F16, tag=f"wd{k}", name=f"wd{k}")
                nc.vector.tensor_sub(wd_k[:], wq_k[:], wk_k[:])
                wd.append(wd_k)
                wv_k = res.tile([128, 128], F16, tag=f"wv{k}", name=f"wv{k}")
                nc.sync.dma_start(wv_k[:], Wvc[k * 128:(k + 1) * 128, :])
                wv.append(wv_k)
                # Wo tiles allocated now, loaded later (DMA deferred past the
                # x loads -- Wo is only needed at the output projection).
                wo_k = res.tile([128, 1024], BF, tag=f"wo{k}", name=f"wo{k}")
                wo.append(wo_k)

            # resident outputs of the projections
            dT = [res.tile([128, 512], F16, tag=f"dT{i}", name=f"dT{i}")
                  for i in range(8)]                  # D^T  [128 feat, 4096 tok]
            Vg = [res.tile([128, 512], BF, tag=f"Vg{i}", name=f"Vg{i}")
                  for i in range(8)]                  # V    [tok, feat] 4 tiles/grp

            # ---------------- projections: D^T and V ----------------
            with tc.tile_pool(name="psp", bufs=1, space="PSUM") as psp:
                for half in range(2):                 # token halves (2048 each)
                    xk_tiles = []
                    for k in range(8):
                        xk = strm.tile([128, 2048], F16, tag="xk", bufs=4,
                                       name=f"xk{half}_{k}")
                        nc.sync.dma_start(
                            xk[:], xT[k * 128:(k + 1) * 128,
                                      half * 2048:(half + 1) * 2048]
                        )
                        xk_tiles.append(xk)
                    pd = [psp.tile([128, 512], FP, tag=f"pd{j}", name=f"pd{half}_{j}")
                          for j in range(4)]
                    pv = [psp.tile([128, 512], FP, tag=f"pv{j}", name=f"pv{half}_{j}")
                          for j in range(4)]
                    for k in range(8):
                        xk = xk_tiles[k]
                        for j in range(4):            # 512-token chunks -> D^T
                            nc.tensor.matmul(
                                pd[j][:], wd[k][:], xk[:, j * 512:(j + 1) * 512],
                                start=(k == 0), stop=(k == 7),
                            )
                        for t in range(16):           # 128-token tiles -> V
                            nc.tensor.matmul(
                                pv[t // 4][:, (t % 4) * 128:(t % 4 + 1) * 128],
                                xk[:, t * 128:(t + 1) * 128], wv[k][:],
                                start=(k == 0 and t % 4 == 0), stop=(k == 7),
                                skip_group_check=True,
                            )
                    for j in range(4):
                        nc.vector.tensor_copy(dT[half * 4 + j][:], pd[j][:])
                        nc.vector.tensor_copy(Vg[half * 4 + j][:], pv[j][:])

            # ---------------- attention (local, 2 heads) ----------------
            # ctx^T accumulates into per-batch SBUF tiles [128, 2048] bf16
            ctxT = [res.tile([128, 2048], BF, tag=f"ctxT{b}", name=f"ctxT{b}")
                    for b in range(B)]

            ctxg = [[], []]                           # [128, 256] bf16 x 8 per b

            def emit_outproj_group(b, fo):
                po = _pso[0].tile([128, 512], FP, tag="po", name=f"po{b}_{fo}")
                for k in range(8):
                    nc.tensor.matmul(
                        po[:, 0:256], wo[k][:, fo * 128:(fo + 1) * 128],
                        ctxg[b][k][:],
                        start=(k == 0), stop=(k == 7),
                    )
                ot = strm.tile([128, 256], FP, tag="ot", bufs=4,
                               name=f"ot{b}_{fo}")
                nc.scalar.activation(
                    ot[:], po[:, 0:256], AF.Identity,
                    bias=bo_t[:, fo:fo + 1], scale=1.0,
                )
                nc.sync.dma_start(
                    out_d[fo * 128:(fo + 1) * 128, b * 256:(b + 1) * 256], ot[:]
                )

            with (
                tc.tile_pool(name="pssc", bufs=2, space="PSUM") as pssc,
                tc.tile_pool(name="pscx", bufs=2, space="PSUM") as pscx,
                tc.tile_pool(name="pso", bufs=2, space="PSUM") as pso,
            ):
                _pso = [pso]
                for b in range(B):
                    for qb in range(NQB):
                        qt = b * 4 + qb               # query dT tile (512 tok)
                        pc = pscx.tile([128, QB], FP, tag="pc", name=f"pc{b}_{qb}")
                        prev = None                   # (at, n, g, m, first, last)
                        for kb in range(4 * qb, NKB):
                            koff = b * S + kb * KB
                            kt, kc = koff // 512, koff % 512
                            n = min(QB, (kb + 1) * KB - qb * QB)
                            j = kb - 4 * qb           # diag sub-block index
                            # -- scores for both heads into one 2-bank tile
                            ps = pssc.tile([128, 1024], FP, tag="ps",
                                           name=f"ps{b}_{qb}_{kb}")
                            for h in range(2):
                                nc.tensor.matmul(
                                    ps[:, 512 * h:512 * h + n],
                                    dT[kt][h * 64:(h + 1) * 64, kc:kc + 128],
                                    dT[qt][h * 64:(h + 1) * 64, 0:n],
                                    start=True, stop=True,
                                )
                            # -- exp on ACT (wide instr; both heads when full)
                            at = strm.tile([128, 1024], BF, tag="at", bufs=6,
                                           name=f"at{b}_{qb}_{kb}")
                            if n == QB:
                                nc.scalar.activation(
                                    at[:], ps[:], AF.Exp, scale=-0.5
                                )
                            else:
                                for h in range(2):
                                    nc.scalar.activation(
                                        at[:, 512 * h:512 * h + n],
                                        ps[:, 512 * h:512 * h + n],
                                        AF.Exp, scale=-0.5,
                                    )
                            if j < 4:                 # diagonal: tril mask on DVE
                                for h in range(2):
                                    lo = 512 * h + 128 * j
                                    nc.vector.tensor_mul(
                                        at[:, lo:lo + 128],
                                        at[:, lo:lo + 128], mask_t[:]
                                    )
                            # -- ctx for the PREVIOUS kb (one-block lag)
                            if prev is not None:
                                _emit_ctx(nc, pc, Vg, prev)
                            prev = (at, n, (b * 16 + kb) // 4, (b * 16 + kb) % 4,
                                    kb == 4 * qb, kb == NKB - 1)
                        _emit_ctx(nc, pc, Vg, prev)
                        nc.vector.tensor_copy(
                            ctxT[b][:, qb * QB:(qb + 1) * QB], pc[:]
                        )

                    if b == 0:
                        # deferred Wo loads drain in the DMA queue's slack
                        # during batch-1 attention
                        for k in range(8):
                            nc.sync.dma_start(
                                wo[k][:], Wob[k * 128:(k + 1) * 128, :]
                            )
                    # AllToAll for this batch (head-shard -> token-shard);
                    # overlaps the next batch's attention on PE/ACT.
                    # Partition-split semantics: slab j of the input goes to
                    # rank j; out slab i arrives from rank i.
                    cc_in = dram.tile([1024, 256], BF, name=f"cc_in{b}")
                    cc_out = dram.tile([1024, 256], BF, name=f"cc_out{b}")
                    for j in range(8):
                        nc.sync.dma_start(
                            cc_in[j * 128:(j + 1) * 128, :],
                            ctxT[b][:, j * 256:(j + 1) * 256],
                        )
                    nc.gpsimd.collective_compute(
                        "AllToAll",
                        mybir.AluOpType.bypass,
                        replica_groups=[list(range(NC))],
                        ins=[cc_in[:].opt()],
                        outs=[cc_out[:].opt()],
                    )
                    if b == 1:
                        # batch-0 output projection overlaps A2A(b1) transfer
                        for fo in range(8):
                            emit_outproj_group(0, fo)
                    for k in range(8):
                        g = res.tile([128, 256], BF, tag=f"cg{b}_{k}",
                                     name=f"cg{b}_{k}")
                        nc.sync.dma_start(g[:], cc_out[k * 128:(k + 1) * 128, :])
                        ctxg[b].append(g)

                for fo in range(8):
                    emit_outproj_group(1, fo)

    nc.compile()
    return nc


def _emit_ctx(nc, pc, Vg, prev):
    at, n, g, m, first, last = prev
    for h in range(2):
        nc.tensor.matmul(
            pc[h * 64:(h + 1) * 64, 0:n],
            Vg[g][:, m * 128 + h * 64:m * 128 + h * 64 + 64],
            at[:, 512 * h:512 * h + n],
            start=first, stop=last,
            skip_group_check=True,
        )


def kernel(**inputs):
    x = np.asarray(inputs["x"], np.float32)
    Wq = np.asarray(inputs["Wq"], np.float32)
    Wk = np.asarray(inputs["Wk"], np.float32)
    Wv = np.asarray(inputs["Wv"], np.float32)
    Wo = np.asarray(inputs["Wo"], np.float32)
    bo = np.asarray(inputs["bo"], np.float32)
    # bq/bk/bv are zeros by the problem's input spec; dif = x @ (Wq - Wk)
    # and v = x @ Wv absorb them exactly when zero.

    if "nc" not in _cached:
        _cached["nc"] = _build()
    nc = _cached["nc"]

    xT = np.ascontiguousarray(x.reshape(T, H).T).astype(np.float16)
    Wob = Wo.astype(ml_dtypes.bfloat16)
    maskbf = np.tril(np.ones((128, 128), np.float32), -1).astype(ml_dtypes.bfloat16)

    in_maps = []
    for c in range(NC):
        cols = slice(c * 128, (c + 1) * 128)
        in_maps.append({
            "xT": xT,
            "Wqc": np.ascontiguousarray(Wq[:, cols]),
            "Wkc": np.ascontiguousarray(Wk[:, cols]),
            "Wvc": np.ascontiguousarray(Wv[:, cols]).astype(np.float16),
            "Wob": Wob,
            "bo": bo,
            "maskbf": maskbf,
        })

    res = run_bass_kernel_spmd(nc, in_maps, core_ids=list(range(NC)))

    out = np.empty((B, S, H), np.float32)
    for c in range(NC):
        oT = res.results[c]["out"]                    # [H, 512]
        for b in range(B):
            out[b, c * 256:(c + 1) * 256, :] = oT[:, b * 256:(b + 1) * 256].T
    return out


# revision 12
# speedup vs baseline: 1.6749x; 1.0446x over previous
"""Distributed gaussian-mask attention for trn2 (8 NeuronCores, SPMD).

Problem: B=2, S=2048, H=1024, 16 heads, hd=64.
  q/k/v = x@W*, dif = q - k, score = exp(-0.5 * dif @ dif^T),
  prob = score * triu(ones,k=1), ctx = prob @ v, out = ctx @ Wo + bo.
  (bq/bk/bv are zeros by construction -- folded out; dif = x @ (Wq-Wk),
   and Wd = Wq-Wk is folded on the host.)

Sharding (uniform SPMD program, data-only per-core differences):
  - Head parallel: core c owns heads (2c, 2c+1) = 128 feature columns of
    Wq/Wk/Wv.  Each core computes D^T = x @ Wd_c and V for ALL tokens of
    its 2 heads, runs the full anti-causal attention triangle locally
    (no collective), producing ctx^T [128, 4096].
  - One AllToAll per batch (partition-split) re-shards ctx from
    head-major to token-major: core c ends with full-H ctx^T for tokens
    [b, 256c:256c+256) of each batch, then does its 1/8 of the output
    projection with the full Wo.

Pipeline structure: the attention inner loop is software-pipelined so
the PE never head-of-line blocks on ACT:
  PE:  score(kb,h0) score(kb,h1) ctx(kb-1,h0) ctx(kb-1,h1) score(kb+1)...
  ACT: exp(kb-1)    exp(kb)      ...   (one [128,1024] instr per kb,
                                        covering both heads)
Scores for both heads of one key block land in one 2-bank PSUM tile;
h0/h1 matmuls pack into disjoint PE row groups (K=64) resp. col groups
(ctx, M=64) and overlap.  Diagonal key blocks use ragged n (per-element
has_written handles the ragged accumulation) + a [128,128] tril mask
multiply on DVE.  Input DMA is split across two queues (sync/vector)
and weight tiles are merged so each needs a single DMA trigger; Wo
loads are deferred to overlap batch-1 attention on the gpsimd queue.

Precision: x/Wd/dT fp16, V/ctx/at/out-proj bf16, PSUM fp32.
"""
import numpy as np
import ml_dtypes

import concourse.bass as bass
import concourse.bacc as bacc
import concourse.mybir as mybir
import concourse.tile as tile
from concourse.tile_rust import add_dep_helper
from concourse.bass_utils import run_bass_kernel_spmd

FP = mybir.dt.float32
F16 = mybir.dt.float16
BF = mybir.dt.bfloat16
AF = mybir.ActivationFunctionType

NC = 8
B, S, H, NH, HD = 2, 2048, 1024, 16, 64
T = B * S            # 4096 tokens
QB = 512             # query block
KB = 128             # key block
NQB = S // QB        # 4 query blocks per batch
NKB = S // KB        # 16 key blocks per batch

_cached = {}


def _ins(x):
    return getattr(x, "ins", x)


def _build():
    nc = bacc.Bacc("TRN2", target_bir_lowering=False, debug=False, num_devices=NC)

    xT = nc.dram_tensor("xT", [H, T], F16, kind="ExternalInput")
    Wdc = nc.dram_tensor("Wdc", [H, 128], F16, kind="ExternalInput")
    Wvc = nc.dram_tensor("Wvc", [H, 128], F16, kind="ExternalInput")
    Wob = nc.dram_tensor("Wob", [H, H], BF, kind="ExternalInput")
    bo_d = nc.dram_tensor("bo", [H], FP, kind="ExternalInput")
    mask_d = nc.dram_tensor("maskbf", [128, 128], BF, kind="ExternalInput")
    out_d = nc.dram_tensor("out", [H, 512], FP, kind="ExternalOutput")

    with tile.TileContext(nc) as tc:
        with (
            tc.tile_pool(name="res", bufs=1) as res,      # resident SBUF
            tc.tile_pool(name="stream", bufs=3) as strm,  # streamed SBUF
            tc.tile_pool(name="dram", bufs=1, space="DRAM") as dram,
        ):
            # ---------------- constants / weights in ----------------
            mask_t = res.tile([128, 128], BF, tag="mask")
            nc.sync.dma_start(mask_t[:], mask_d[:])
            bo_t = res.tile([128, 8], FP, tag="bo")
            nc.sync.dma_start(bo_t[:], bo_d[:].rearrange("(f p) -> p f", p=128))

            # merged weight tiles: col block k = rows [128k, 128k+128) of W
            wd_t = res.tile([128, 1024], F16, tag="wd")
            nc.sync.dma_start(
                wd_t[:].rearrange("p (k m) -> p k m", k=8),
                Wdc[:].rearrange("(k p) m -> p k m", p=128),
            )
            wv_t = res.tile([128, 1024], F16, tag="wv")
            nc.sync.dma_start(
                wv_t[:].rearrange("p (k m) -> p k m", k=8),
                Wvc[:].rearrange("(k p) m -> p k m", p=128),
            )
            # Wo allocated now, loaded later on the gpsimd DMA queue (only
            # needed at the output projection)
            wo_t = res.tile([128, 8192], BF, tag="wo")

            # resident outputs of the projections
            dT = [res.tile([128, 512], F16, tag=f"dT{i}", name=f"dT{i}")
                  for i in range(8)]                  # D^T  [128 feat, 4096 tok]
            Vg = [res.tile([128, 512], BF, tag=f"Vg{i}", name=f"Vg{i}")
                  for i in range(8)]                  # V    [tok, feat] 4 tiles/grp

            # ---------------- projections: D^T and V ----------------
            with tc.tile_pool(name="psp", bufs=1, space="PSUM") as psp:
                for half in range(2):                 # token halves (2048 each)
                    xk_tiles = []
                    for k in range(8):
                        xk = strm.tile([128, 2048], F16, tag="xk", bufs=4,
                                       name=f"xk{half}_{k}")
                        eng = nc.sync if k % 2 == 0 else nc.scalar
                        eng.dma_start(
                            xk[:], xT[k * 128:(k + 1) * 128,
                                      half * 2048:(half + 1) * 2048]
                        )
                        xk_tiles.append(xk)
                    pd = [psp.tile([128, 512], FP, tag=f"pd{j}", name=f"pd{half}_{j}")
                          for j in range(4)]
                    pv = [psp.tile([128, 512], FP, tag=f"pv{j}", name=f"pv{half}_{j}")
                          for j in range(4)]
                    for k in range(8):
                        xk = xk_tiles[k]
                        for j in range(4):            # 512-token chunks -> D^T
                            nc.tensor.matmul(
                                pd[j][:], wd_t[:, k * 128:(k + 1) * 128],
                                xk[:, j * 512:(j + 1) * 512],
                                start=(k == 0), stop=(k == 7),
                            )
                        for t in range(16):           # 128-token tiles -> V
                            nc.tensor.matmul(
                                pv[t // 4][:, (t % 4) * 128:(t % 4 + 1) * 128],
                                xk[:, t * 128:(t + 1) * 128],
                                wv_t[:, k * 128:(k + 1) * 128],
                                start=(k == 0 and t % 4 == 0), stop=(k == 7),
                                skip_group_check=True,
                            )
                    for j in range(4):
                        nc.vector.tensor_copy(dT[half * 4 + j][:], pd[j][:])
                        nc.vector.tensor_copy(Vg[half * 4 + j][:], pv[j][:])

            # ---------------- attention (local, 2 heads) ----------------
            # ctx^T accumulates into per-batch SBUF tiles [128, 2048] bf16
            ctxT = [res.tile([128, 2048], BF, tag=f"ctxT{b}", name=f"ctxT{b}")
                    for b in range(B)]

            ctxg = [None, None]                       # [128, 2048] bf16 per b

            def emit_outproj_group(b, fo):
                po = _pso[0].tile([128, 512], FP, tag="po", name=f"po{b}_{fo}")
                first = None
                for k in range(8):
                    mm = nc.tensor.matmul(
                        po[:, 0:256],
                        wo_t[:, k * 1024 + fo * 128:k * 1024 + (fo + 1) * 128],
                        ctxg[b][:, k * 256:(k + 1) * 256],
                        start=(k == 0), stop=(k == 7),
                    )
                    if first is None:
                        first = mm
                ot = strm.tile([128, 256], FP, tag="ot", bufs=4,
                               name=f"ot{b}_{fo}")
                nc.scalar.activation(
                    ot[:], po[:, 0:256], AF.Identity,
                    bias=bo_t[:, fo:fo + 1], scale=1.0,
                )
                nc.sync.dma_start(
                    out_d[fo * 128:(fo + 1) * 128, b * 256:(b + 1) * 256], ot[:]
                )
                return first

            last_ctx = [None]

            with (
                tc.tile_pool(name="pssc", bufs=2, space="PSUM") as pssc,
                tc.tile_pool(name="pscx", bufs=2, space="PSUM") as pscx,
                tc.tile_pool(name="pso", bufs=2, space="PSUM") as pso,
            ):
                _pso = [pso]
                for b in range(B):
                    for qb in range(NQB):
                        qt = b * 4 + qb               # query dT tile (512 tok)
                        pc = pscx.tile([128, QB], FP, tag="pc", name=f"pc{b}_{qb}")
                        prev = None
                        for kb in range(4 * qb, NKB):
                            koff = b * S + kb * KB
                            kt, kc = koff // 512, koff % 512
                            n = min(QB, (kb + 1) * KB - qb * QB)
                            j = kb - 4 * qb           # diag sub-block index
                            # -- scores for both heads into one 2-bank tile
                            ps = pssc.tile([128, 1024], FP, tag="ps",
                                           name=f"ps{b}_{qb}_{kb}")
                            for h in range(2):
                                nc.tensor.matmul(
                                    ps[:, 512 * h:512 * h + n],
                                    dT[kt][h * 64:(h + 1) * 64, kc:kc + 128],
                                    dT[qt][h * 64:(h + 1) * 64, 0:n],
                                    start=True, stop=True,
                                )
                            # -- exp on ACT (wide instr; both heads when full)
                            at = strm.tile([128, 1024], BF, tag="at", bufs=6,
                                           name=f"at{b}_{qb}_{kb}")
                            if n == QB:
                                nc.scalar.activation(
                                    at[:], ps[:], AF.Exp, scale=-0.5
                                )
                            else:
                                for h in range(2):
                                    nc.scalar.activation(
                                        at[:, 512 * h:512 * h + n],
                                        ps[:, 512 * h:512 * h + n],
                                        AF.Exp, scale=-0.5,
                                    )
                            if j < 4:                 # diagonal: tril mask on DVE
                                for h in range(2):
                                    lo = 512 * h + 128 * j
                                    nc.vector.tensor_mul(
                                        at[:, lo:lo + 128],
                                        at[:, lo:lo + 128], mask_t[:]
                                    )
                            # -- ctx for the PREVIOUS kb (one-block lag)
                            if prev is not None:
                                _emit_ctx(nc, pc, Vg, prev)
                            prev = (at, n, (b * 16 + kb) // 4, (b * 16 + kb) % 4,
                                    kb == 4 * qb, kb == NKB - 1)
                        last_ctx[0] = _emit_ctx(nc, pc, Vg, prev)
                        nc.vector.tensor_copy(
                            ctxT[b][:, qb * QB:(qb + 1) * QB], pc[:]
                        )

                    if b == 0:
                        # deferred Wo load drains on the gpsimd DMA queue
                        # during batch-1 attention
                        nc.gpsimd.dma_start(
                            wo_t[:].rearrange("p (k m) -> p k m", k=8),
                            Wob[:].rearrange("(k p) m -> p k m", p=128),
                        )
                    # AllToAll for this batch (head-shard -> token-shard);
                    # overlaps the next batch's attention on PE/ACT.
                    # Partition-split semantics: slab j of the input goes to
                    # rank j; out slab i arrives from rank i.
                    cc_in = dram.tile([1024, 256], BF, name=f"cc_in{b}")
                    cc_out = dram.tile([1024, 256], BF, name=f"cc_out{b}")
                    nc.sync.dma_start(
                        cc_in[:].rearrange("(j p) c -> p j c", p=128),
                        ctxT[b][:].rearrange("p (j c) -> p j c", j=8),
                    )
                    nc.gpsimd.collective_compute(
                        "AllToAll",
                        mybir.AluOpType.bypass,
                        replica_groups=[list(range(NC))],
                        ins=[cc_in[:].opt()],
                        outs=[cc_out[:].opt()],
                    )
                    g = res.tile([128, 2048], BF, tag=f"cg{b}", name=f"cg{b}")
                    nc.sync.dma_start(
                        g[:].rearrange("p (k c) -> p k c", k=8),
                        cc_out[:].rearrange("(k p) c -> p k c", p=128),
                    )
                    ctxg[b] = g
                    if b == 1:
                        # batch-0 output projection overlaps A2A(b1); the
                        # explicit dep keeps it from preempting b1 attention
                        # at the head of the PE queue.
                        first = emit_outproj_group(0, 0)
                        add_dep_helper(
                            _ins(first), _ins(last_ctx[0]), sync=False,
                            reason="outproj(b0) after b1 attention",
                        )
                        for fo in range(1, 8):
                            emit_outproj_group(0, fo)

                for fo in range(8):
                    emit_outproj_group(1, fo)

    nc.compile()
    return nc


def _emit_ctx(nc, pc, Vg, prev):
    at, n, g, m, first, last = prev
    mm = None
    for h in range(2):
        mm = nc.tensor.matmul(
            pc[h * 64:(h + 1) * 64, 0:n],
            Vg[g][:, m * 128 + h * 64:m * 128 + h * 64 + 64],
            at[:, 512 * h:512 * h + n],
            start=first, stop=last,
            skip_group_check=True,
        )
    return mm


def make_in_maps(inputs):
    x = np.asarray(inputs["x"], np.float32)
    Wq = np.asarray(inputs["Wq"], np.float32)
    Wk = np.asarray(inputs["Wk"], np.float32)
    Wv = np.asarray(inputs["Wv"], np.float32)
    Wo = np.asarray(inputs["Wo"], np.float32)
    bo = np.asarray(inputs["bo"], np.float32)
    # bq/bk/bv are zeros by the problem's input spec; dif = x @ (Wq - Wk)
    # and v = x @ Wv absorb them exactly when zero.

    xT = np.ascontiguousarray(x.reshape(T, H).T).astype(np.float16)
    Wd = (Wq - Wk).astype(np.float16)
    Wob = Wo.astype(ml_dtypes.bfloat16)
    maskbf = np.tril(np.ones((128, 128), np.float32), -1).astype(ml_dtypes.bfloat16)

    in_maps = []
    for c in range(NC):
        cols = slice(c * 128, (c + 1) * 128)
        in_maps.append({
            "xT": xT,
            "Wdc": np.ascontiguousarray(Wd[:, cols]),
            "Wvc": np.ascontiguousarray(Wv[:, cols]).astype(np.float16),
            "Wob": Wob,
            "bo": bo,
            "maskbf": maskbf,
        })
    return in_maps


def kernel(**inputs):
    if "nc" not in _cached:
        _cached["nc"] = _build()
    nc = _cached["nc"]

    res = run_bass_kernel_spmd(nc, make_in_maps(inputs), core_ids=list(range(NC)))

    out = np.empty((B, S, H), np.float32)
    for c in range(NC):
        oT = res.results[c]["out"]                    # [H, 512]
        for b in range(B):
            out[b, c * 256:(c + 1) * 256, :] = oT[:, b * 256:(b + 1) * 256].T
    return out


# revision 13
# speedup vs baseline: 1.9170x; 1.1445x over previous
"""Distributed gaussian-mask attention for trn2 (8 NeuronCores, SPMD).

Problem: B=2, S=2048, H=1024, 16 heads, hd=64.
  q/k/v = x@W*, dif = q - k, score = exp(-0.5 * dif @ dif^T),
  prob = score * triu(ones,k=1), ctx = prob @ v, out = ctx @ Wo + bo.
  (bq/bk/bv are zeros by construction -- folded out; dif = x @ (Wq-Wk),
   and Wd = Wq-Wk is folded on the host.)

Sharding (uniform SPMD program, data-only per-core differences):
  - Head parallel: core c owns heads (2c, 2c+1) = 128 feature columns of
    Wq/Wk/Wv.  Each core computes D^T = x @ Wd_c and V for ALL tokens of
    its 2 heads, runs the full anti-causal attention triangle locally
    (no collective), producing ctx^T [128, 4096].
  - One AllToAll per batch (partition-split) re-shards ctx from
    head-major to token-major: core c ends with full-H ctx^T for tokens
    [b, 256c:256c+256) of each batch, then does its 1/8 of the output
    projection with the full Wo.

Pipeline structure: projections are emitted per 512-token quarter and
interleaved with the attention loops, so batch-0 attention starts as
soon as quarter 0 of D^T/V is done (~1MB of x loaded) instead of after
the full 8MB.  The attention inner loop is software-pipelined so the PE
never head-of-line blocks on ACT:
  PE:  score(kb,h0) score(kb,h1) ctx(kb-1,h0) ctx(kb-1,h1) score(kb+1)...
  ACT: exp(kb-1)    exp(kb)      ...   (one [128,1024] instr per kb,
                                        covering both heads)
Scores for both heads of one key block land in one 2-bank PSUM tile;
h0/h1 matmuls pack into disjoint PE row groups (K=64) resp. col groups
(ctx, M=64) and overlap.  Diagonal key blocks use ragged n (per-element
has_written handles the ragged accumulation) + a [128,128] tril mask
multiply on DVE.  Input DMA is split across two queues (sync/scalar);
Wo loads are deferred to overlap batch-1 attention.

Precision: x/Wd/dT fp16, V/ctx/at/out-proj bf16, PSUM fp32.
"""
import numpy as np
import ml_dtypes

import concourse.bass as bass
import concourse.bacc as bacc
import concourse.mybir as mybir
import concourse.tile as tile
from concourse.tile_rust import add_dep_helper
from concourse.bass_utils import run_bass_kernel_spmd

FP = mybir.dt.float32
F16 = mybir.dt.float16
BF = mybir.dt.bfloat16
AF = mybir.ActivationFunctionType

NC = 8
B, S, H, NH, HD = 2, 2048, 1024, 16, 64
T = B * S            # 4096 tokens
QB = 512             # query block
KB = 128             # key block
NQB = S // QB        # 4 query blocks per batch
NKB = S // KB        # 16 key blocks per batch

_cached = {}


def _ins(x):
    return getattr(x, "ins", x)


def _build():
    nc = bacc.Bacc("TRN2", target_bir_lowering=False, debug=False, num_devices=NC)

    xT = nc.dram_tensor("xT", [H, T], F16, kind="ExternalInput")
    Wdc = nc.dram_tensor("Wdc", [H, 128], F16, kind="ExternalInput")
    Wvc = nc.dram_tensor("Wvc", [H, 128], F16, kind="ExternalInput")
    Wob = nc.dram_tensor("Wob", [H, H], BF, kind="ExternalInput")
    bo_d = nc.dram_tensor("bo", [H], FP, kind="ExternalInput")
    mask_d = nc.dram_tensor("maskbf", [128, 128], BF, kind="ExternalInput")
    out_d = nc.dram_tensor("out", [H, 512], FP, kind="ExternalOutput")

    with tile.TileContext(nc) as tc:
        with (
            tc.tile_pool(name="res", bufs=1) as res,      # resident SBUF
            tc.tile_pool(name="stream", bufs=3) as strm,  # streamed SBUF
            tc.tile_pool(name="dram", bufs=1, space="DRAM") as dram,
            tc.tile_pool(name="psp", bufs=1, space="PSUM") as psp,
            tc.tile_pool(name="pssc", bufs=2, space="PSUM") as pssc,
            tc.tile_pool(name="pscx", bufs=1, space="PSUM") as pscx,
            tc.tile_pool(name="pso", bufs=1, space="PSUM") as pso,
        ):
            # ---------------- constants / weights in ----------------
            mask_t = res.tile([128, 128], BF, tag="mask")
            nc.sync.dma_start(mask_t[:], mask_d[:])
            bo_t = res.tile([128, 8], FP, tag="bo")
            nc.sync.dma_start(bo_t[:], bo_d[:].rearrange("(f p) -> p f", p=128))

            # merged weight tiles: col block k = rows [128k, 128k+128) of W
            wd_t = res.tile([128, 1024], F16, tag="wd")
            nc.sync.dma_start(
                wd_t[:].rearrange("p (k m) -> p k m", k=8),
                Wdc[:].rearrange("(k p) m -> p k m", p=128),
            )
            wv_t = res.tile([128, 1024], F16, tag="wv")
            nc.sync.dma_start(
                wv_t[:].rearrange("p (k m) -> p k m", k=8),
                Wvc[:].rearrange("(k p) m -> p k m", p=128),
            )
            # Wo allocated now, loaded later (only needed at out-proj)
            wo_t = res.tile([128, 8192], BF, tag="wo")

            # resident outputs of the projections (per 512-token quarter)
            dT = [res.tile([128, 512], F16, tag=f"dT{i}", name=f"dT{i}")
                  for i in range(8)]                  # D^T  [128 feat, 4096 tok]
            Vg = [res.tile([128, 512], BF, tag=f"Vg{i}", name=f"Vg{i}")
                  for i in range(8)]                  # V    [tok, feat] 4 tiles/grp

            def emit_proj_quarter(q):
                """D^T and V for tokens [512q, 512q+512)."""
                xq = []
                for k in range(8):
                    x = strm.tile([128, 512], F16, tag="xq", bufs=16,
                                  name=f"xq{q}_{k}")
                    eng = nc.sync if k % 2 == 0 else nc.scalar
                    eng.dma_start(
                        x[:], xT[k * 128:(k + 1) * 128,
                                 q * 512:(q + 1) * 512]
                    )
                    xq.append(x)
                pd = psp.tile([128, 512], FP, tag="pd", name=f"pd{q}")
                for k in range(8):
                    nc.tensor.matmul(
                        pd[:], wd_t[:, k * 128:(k + 1) * 128], xq[k][:],
                        start=(k == 0), stop=(k == 7),
                    )
                nc.vector.tensor_copy(dT[q][:], pd[:])
                pv = psp.tile([128, 512], FP, tag="pv", name=f"pv{q}")
                for k in range(8):
                    for t in range(4):                # 128-token tiles -> V
                        nc.tensor.matmul(
                            pv[:, t * 128:(t + 1) * 128],
                            xq[k][:, t * 128:(t + 1) * 128],
                            wv_t[:, k * 128:(k + 1) * 128],
                            start=(k == 0 and t == 0), stop=(k == 7),
                            skip_group_check=True,
                        )
                nc.vector.tensor_copy(Vg[q][:], pv[:])

            # ---------------- attention (local, 2 heads) ----------------
            # ctx^T accumulates into per-batch SBUF tiles [128, 2048] bf16
            ctxT = [res.tile([128, 2048], BF, tag=f"ctxT{b}", name=f"ctxT{b}")
                    for b in range(B)]

            ctxg = [None, None]                       # [128, 2048] bf16 per b

            def emit_outproj_group(b, fo):
                po = pso.tile([128, 512], FP, tag="po", name=f"po{b}_{fo}")
                first = None
                for k in range(8):
                    mm = nc.tensor.matmul(
                        po[:, 0:256],
                        wo_t[:, k * 1024 + fo * 128:k * 1024 + (fo + 1) * 128],
                        ctxg[b][:, k * 256:(k + 1) * 256],
                        start=(k == 0), stop=(k == 7),
                    )
                    if first is None:
                        first = mm
                ot = strm.tile([128, 256], FP, tag="ot", bufs=4,
                               name=f"ot{b}_{fo}")
                nc.scalar.activation(
                    ot[:], po[:, 0:256], AF.Identity,
                    bias=bo_t[:, fo:fo + 1], scale=1.0,
                )
                nc.sync.dma_start(
                    out_d[fo * 128:(fo + 1) * 128, b * 256:(b + 1) * 256], ot[:]
                )
                return first

            def emit_attn_qblock(b, qb):
                qt = b * 4 + qb                       # query dT tile (512 tok)
                pc = pscx.tile([128, QB], FP, tag="pc", name=f"pc{b}_{qb}")
                prev = None
                for kb in range(4 * qb, NKB):
                    koff = b * S + kb * KB
                    kt, kc = koff // 512, koff % 512
                    n = min(QB, (kb + 1) * KB - qb * QB)
                    j = kb - 4 * qb                   # diag sub-block index
                    # -- scores for both heads into one 2-bank tile
                    ps = pssc.tile([128, 1024], FP, tag="ps",
                                   name=f"ps{b}_{qb}_{kb}")
                    for h in range(2):
                        nc.tensor.matmul(
                            ps[:, 512 * h:512 * h + n],
                            dT[kt][h * 64:(h + 1) * 64, kc:kc + 128],
                            dT[qt][h * 64:(h + 1) * 64, 0:n],
                            start=True, stop=True,
                        )
                    # -- exp on ACT (one wide instr per kb, both heads)
                    at = strm.tile([128, 1024], BF, tag="at", bufs=6,
                                   name=f"at{b}_{qb}_{kb}")
                    if n == QB:
                        nc.scalar.activation(
                            at[:], ps[:], AF.Exp, scale=-0.5
                        )
                    else:
                        nc.scalar.activation(
                            at[:].rearrange("p (h c) -> p h c", h=2)[:, :, 0:n],
                            ps[:].rearrange("p (h c) -> p h c", h=2)[:, :, 0:n],
                            AF.Exp, scale=-0.5,
                        )
                    if j < 4:                         # diagonal: tril mask, DVE
                        for h in range(2):
                            lo = 512 * h + 128 * j
                            nc.vector.tensor_mul(
                                at[:, lo:lo + 128],
                                at[:, lo:lo + 128], mask_t[:]
                            )
                    # -- ctx for the PREVIOUS kb (one-block lag)
                    if prev is not None:
                        _emit_ctx(nc, pc, Vg, prev)
                    prev = (at, n, (b * 16 + kb) // 4, (b * 16 + kb) % 4,
                            kb == 4 * qb, kb == NKB - 1)
                last = _emit_ctx(nc, pc, Vg, prev)
                nc.vector.tensor_copy(
                    ctxT[b][:, qb * QB:(qb + 1) * QB], pc[:]
                )
                return last

            def emit_a2a(b):
                cc_in = dram.tile([1024, 256], BF, name=f"cc_in{b}")
                cc_out = dram.tile([1024, 256], BF, name=f"cc_out{b}")
                nc.sync.dma_start(
                    cc_in[:].rearrange("(j p) c -> p j c", p=128),
                    ctxT[b][:].rearrange("p (j c) -> p j c", j=8),
                )
                nc.gpsimd.collective_compute(
                    "AllToAll",
                    mybir.AluOpType.bypass,
                    replica_groups=[list(range(NC))],
                    ins=[cc_in[:].opt()],
                    outs=[cc_out[:].opt()],
                )
                g = res.tile([128, 2048], BF, tag=f"cg{b}", name=f"cg{b}")
                nc.sync.dma_start(
                    g[:].rearrange("p (k c) -> p k c", k=8),
                    cc_out[:].rearrange("(k p) c -> p k c", p=128),
                )
                ctxg[b] = g

            # quarter-granular projections interleaved with attention:
            # b0-qb blocks need dT/Vg[0..3]; b1 needs [4..7].
            for q in range(4):
                emit_proj_quarter(q)
            emit_attn_qblock(0, 0)
            emit_proj_quarter(4)
            emit_attn_qblock(0, 1)
            emit_proj_quarter(5)
            emit_attn_qblock(0, 2)
            emit_proj_quarter(6)
            emit_attn_qblock(0, 3)
            emit_proj_quarter(7)
            # deferred Wo load drains in DMA-queue slack from here on
            nc.sync.dma_start(
                wo_t[:].rearrange("p (k m) -> p k m", k=8),
                Wob[:].rearrange("(k p) m -> p k m", p=128),
            )
            emit_a2a(0)

            last_b1 = None
            for qb in range(NQB):
                last_b1 = emit_attn_qblock(1, qb)
            emit_a2a(1)

            # batch-0 output projection overlaps A2A(b1); the explicit dep
            # keeps it from preempting b1 attention at the PE queue head.
            first = emit_outproj_group(0, 0)
            add_dep_helper(
                _ins(first), _ins(last_b1), sync=False,
                reason="outproj(b0) after b1 attention",
            )
            for fo in range(1, 8):
                emit_outproj_group(0, fo)
            for fo in range(8):
                emit_outproj_group(1, fo)

    nc.compile()
    return nc


def _emit_ctx(nc, pc, Vg, prev):
    at, n, g, m, first, last = prev
    mm = None
    for h in range(2):
        mm = nc.tensor.matmul(
            pc[h * 64:(h + 1) * 64, 0:n],
            Vg[g][:, m * 128 + h * 64:m * 128 + h * 64 + 64],
            at[:, 512 * h:512 * h + n],
            start=first, stop=last,
            skip_group_check=True,
        )
    return mm


def make_in_maps(inputs):
    x = np.asarray(inputs["x"], np.float32)
    Wq = np.asarray(inputs["Wq"], np.float32)
    Wk = np.asarray(inputs["Wk"], np.float32)
    Wv = np.asarray(inputs["Wv"], np.float32)
    Wo = np.asarray(inputs["Wo"], np.float32)
    bo = np.asarray(inputs["bo"], np.float32)
    # bq/bk/bv are zeros by the problem's input spec; dif = x @ (Wq - Wk)
    # and v = x @ Wv absorb them exactly when zero.

    xT = np.ascontiguousarray(x.reshape(T, H).T).astype(np.float16)
    Wd = (Wq - Wk).astype(np.float16)
    Wob = Wo.astype(ml_dtypes.bfloat16)
    maskbf = np.tril(np.ones((128, 128), np.float32), -1).astype(ml_dtypes.bfloat16)

    in_maps = []
    for c in range(NC):
        cols = slice(c * 128, (c + 1) * 128)
        in_maps.append({
            "xT": xT,
            "Wdc": np.ascontiguousarray(Wd[:, cols]),
            "Wvc": np.ascontiguousarray(Wv[:, cols]).astype(np.float16),
            "Wob": Wob,
            "bo": bo,
            "maskbf": maskbf,
        })
    return in_maps


def kernel(**inputs):
    if "nc" not in _cached:
        _cached["nc"] = _build()
    nc = _cached["nc"]

    res = run_bass_kernel_spmd(nc, make_in_maps(inputs), core_ids=list(range(NC)))

    out = np.empty((B, S, H), np.float32)
    for c in range(NC):
        oT = res.results[c]["out"]                    # [H, 512]
        for b in range(B):
            out[b, c * 256:(c + 1) * 256, :] = oT[:, b * 256:(b + 1) * 256].T
    return out


# revision 17
# speedup vs baseline: 1.9443x; 1.0142x over previous
"""Distributed gaussian-mask attention for trn2 (8 NeuronCores, SPMD).

Problem: B=2, S=2048, H=1024, 16 heads, hd=64.
  q/k/v = x@W*, dif = q - k, score = exp(-0.5 * dif @ dif^T),
  prob = score * triu(ones,k=1), ctx = prob @ v, out = ctx @ Wo + bo.
  (bq/bk/bv are zeros by construction -- folded out; dif = x @ (Wq-Wk),
   and Wd = Wq-Wk is folded on the host.)

Sharding (uniform SPMD program, data-only per-core differences):
  - Head parallel: core c owns heads (2c, 2c+1) = 128 feature columns of
    Wq/Wk/Wv.  Each core computes D^T = x @ Wd_c and V for ALL tokens of
    its 2 heads, runs the full anti-causal attention triangle locally
    (no collective), producing ctx^T [128, 4096].
  - One AllToAll per batch (partition-split) re-shards ctx from
    head-major to token-major: core c ends with full-H ctx^T for tokens
    [b, 256c:256c+256) of each batch, then does its 1/8 of the output
    projection with the full Wo.

Pipeline structure: projections are emitted per 512-token quarter and
interleaved with the attention loops, so batch-0 attention starts as
soon as quarter 0 of D^T/V is done (~1MB of x loaded) instead of after
the full 8MB.  The attention inner loop is software-pipelined so the PE
never head-of-line blocks on ACT:
  PE:  score(kb,h0) score(kb,h1) ctx(kb-1,h0) ctx(kb-1,h1) score(kb+1)...
  ACT: exp(kb-1)    exp(kb)      ...   (one [128,1024] instr per kb,
                                        covering both heads)
Scores for both heads of one key block land in one 2-bank PSUM tile;
h0/h1 matmuls pack into disjoint PE row groups (K=64) resp. col groups
(ctx, M=64) and overlap.  Diagonal key blocks use ragged n (per-element
has_written handles the ragged accumulation) + a [128,128] tril mask
multiply on DVE.  Input DMA is split across two queues (sync/scalar);
Wo loads are deferred to overlap batch-1 attention.

Precision: x/Wd/dT fp16, V/ctx/at/out-proj bf16, PSUM fp32.
"""
import numpy as np
import ml_dtypes

import concourse.bass as bass
import concourse.bacc as bacc
import concourse.mybir as mybir
import concourse.tile as tile
from concourse.tile_rust import add_dep_helper
from concourse.bass_utils import run_bass_kernel_spmd

FP = mybir.dt.float32
F16 = mybir.dt.float16
BF = mybir.dt.bfloat16
AF = mybir.ActivationFunctionType

NC = 8
B, S, H, NH, HD = 2, 2048, 1024, 16, 64
T = B * S            # 4096 tokens
QB = 512             # query block
KB = 128             # key block
NQB = S // QB        # 4 query blocks per batch
NKB = S // KB        # 16 key blocks per batch

_cached = {}


def _ins(x):
    return getattr(x, "ins", x)


def _build():
    nc = bacc.Bacc("TRN2", target_bir_lowering=False, debug=False, num_devices=NC)

    xT = nc.dram_tensor("xT", [H, T], F16, kind="ExternalInput")
    Wdc = nc.dram_tensor("Wdc", [H, 128], F16, kind="ExternalInput")
    Wvc = nc.dram_tensor("Wvc", [H, 128], F16, kind="ExternalInput")
    Wob = nc.dram_tensor("Wob", [H, H], BF, kind="ExternalInput")
    bo_d = nc.dram_tensor("bo", [H], FP, kind="ExternalInput")
    mask_d = nc.dram_tensor("maskbf", [128, 128], BF, kind="ExternalInput")
    out_d = nc.dram_tensor("out", [H, 512], FP, kind="ExternalOutput")

    with tile.TileContext(nc) as tc:
        with (
            tc.tile_pool(name="res", bufs=1) as res,      # resident SBUF
            tc.tile_pool(name="stream", bufs=3) as strm,  # streamed SBUF
            tc.tile_pool(name="dram", bufs=1, space="DRAM") as dram,
            tc.tile_pool(name="psp", bufs=1, space="PSUM") as psp,
            tc.tile_pool(name="pssc", bufs=2, space="PSUM") as pssc,
            tc.tile_pool(name="pscx", bufs=1, space="PSUM") as pscx,
        ):
            # ---------------- constants / weights in ----------------
            mask_t = res.tile([128, 128], BF, tag="mask")
            nc.sync.dma_start(mask_t[:], mask_d[:])
            bo_t = res.tile([128, 8], FP, tag="bo")
            nc.sync.dma_start(bo_t[:], bo_d[:].rearrange("(f p) -> p f", p=128))

            # merged weight tiles: col block k = rows [128k, 128k+128) of W
            wd_t = res.tile([128, 1024], F16, tag="wd")
            nc.sync.dma_start(
                wd_t[:].rearrange("p (k m) -> p k m", k=8),
                Wdc[:].rearrange("(k p) m -> p k m", p=128),
            )
            wv_t = res.tile([128, 1024], F16, tag="wv")
            nc.sync.dma_start(
                wv_t[:].rearrange("p (k m) -> p k m", k=8),
                Wvc[:].rearrange("(k p) m -> p k m", p=128),
            )
            # Wo allocated now, loaded later (only needed at out-proj)
            wo_t = res.tile([128, 8192], BF, tag="wo")

            # resident outputs of the projections (per 512-token quarter)
            dT = [res.tile([128, 512], F16, tag=f"dT{i}", name=f"dT{i}")
                  for i in range(8)]                  # D^T  [128 feat, 4096 tok]
            Vg = [res.tile([128, 512], BF, tag=f"Vg{i}", name=f"Vg{i}")
                  for i in range(8)]                  # V    [tok, feat] 4 tiles/grp

            xq_tiles = {}

            def emit_proj_d(q):
                """x loads + D^T for tokens [512q, 512q+512)."""
                xq = []
                for k in range(8):
                    x = strm.tile([128, 512], F16, tag="xq", bufs=24,
                                  name=f"xq{q}_{k}")
                    eng = nc.sync if k % 2 == 0 else nc.scalar
                    eng.dma_start(
                        x[:], xT[k * 128:(k + 1) * 128,
                                 q * 512:(q + 1) * 512]
                    )
                    xq.append(x)
                xq_tiles[q] = xq
                pd = psp.tile([128, 512], FP, tag="pd", bufs=2, name=f"pd{q}")
                for k in range(8):
                    nc.tensor.matmul(
                        pd[:], wd_t[:, k * 128:(k + 1) * 128], xq[k][:],
                        start=(k == 0), stop=(k == 7),
                    )
                nc.vector.tensor_copy(dT[q][:], pd[:])

            def emit_proj_v(q):
                """V for tokens [512q, 512q+512)."""
                xq = xq_tiles.pop(q)
                pv = psp.tile([128, 512], FP, tag="pv", name=f"pv{q}")
                for k in range(8):
                    for t in range(4):                # 128-token tiles -> V
                        nc.tensor.matmul(
                            pv[:, t * 128:(t + 1) * 128],
                            xq[k][:, t * 128:(t + 1) * 128],
                            wv_t[:, k * 128:(k + 1) * 128],
                            start=(k == 0 and t == 0), stop=(k == 7),
                            skip_group_check=True,
                        )
                nc.vector.tensor_copy(Vg[q][:], pv[:])

            # ---------------- attention (local, 2 heads) ----------------
            # ctx^T accumulates into per-batch SBUF tiles [128, 2048] bf16
            ctxT = [res.tile([128, 2048], BF, tag=f"ctxT{b}", name=f"ctxT{b}")
                    for b in range(B)]

            ctxg = [None, None]                       # [128, 2048] bf16 per b

            def emit_outproj_group(b, fo):
                # shares the (2-bank) "ps" slots -> double-buffered out-proj
                po = pssc.tile([128, 512], FP, tag="ps", name=f"po{b}_{fo}")
                first = None
                for k in range(8):
                    mm = nc.tensor.matmul(
                        po[:, 0:256],
                        wo_t[:, k * 1024 + fo * 128:k * 1024 + (fo + 1) * 128],
                        ctxg[b][:, k * 256:(k + 1) * 256],
                        start=(k == 0), stop=(k == 7),
                    )
                    if first is None:
                        first = mm
                ot = strm.tile([128, 256], FP, tag="ot", bufs=4,
                               name=f"ot{b}_{fo}")
                nc.scalar.activation(
                    ot[:], po[:, 0:256], AF.Identity,
                    bias=bo_t[:, fo:fo + 1], scale=1.0,
                )
                nc.sync.dma_start(
                    out_d[fo * 128:(fo + 1) * 128, b * 256:(b + 1) * 256], ot[:]
                )
                return first

            def emit_attn_qblock(b, qb):
                qt = b * 4 + qb                       # query dT tile (512 tok)
                pc = pscx.tile([128, QB], FP, tag="pc", name=f"pc{b}_{qb}")
                prev = None
                for kb in range(4 * qb, NKB):
                    koff = b * S + kb * KB
                    kt, kc = koff // 512, koff % 512
                    n = min(QB, (kb + 1) * KB - qb * QB)
                    j = kb - 4 * qb                   # diag sub-block index
                    # -- scores for both heads into one 2-bank tile
                    ps = pssc.tile([128, 1024], FP, tag="ps",
                                   name=f"ps{b}_{qb}_{kb}")
                    for h in range(2):
                        nc.tensor.matmul(
                            ps[:, 512 * h:512 * h + n],
                            dT[kt][h * 64:(h + 1) * 64, kc:kc + 128],
                            dT[qt][h * 64:(h + 1) * 64, 0:n],
                            start=True, stop=True,
                        )
                    # -- exp on ACT (one wide instr per kb, both heads)
                    at = strm.tile([128, 1024], BF, tag="at", bufs=6,
                                   name=f"at{b}_{qb}_{kb}")
                    if n == QB:
                        nc.scalar.activation(
                            at[:], ps[:], AF.Exp, scale=-0.5
                        )
                    else:
                        nc.scalar.activation(
                            at[:].rearrange("p (h c) -> p h c", h=2)[:, :, 0:n],
                            ps[:].rearrange("p (h c) -> p h c", h=2)[:, :, 0:n],
                            AF.Exp, scale=-0.5,
                        )
                    if j < 4:                         # diagonal: tril mask, DVE
                        for h in range(2):
                            lo = 512 * h + 128 * j
                            nc.vector.tensor_mul(
                                at[:, lo:lo + 128],
                                at[:, lo:lo + 128], mask_t[:]
                            )
                    # -- ctx for the PREVIOUS kb (one-block lag)
                    if prev is not None:
                        _emit_ctx(nc, pc, Vg, prev)
                    prev = (at, n, (b * 16 + kb) // 4, (b * 16 + kb) % 4,
                            kb == 4 * qb, kb == NKB - 1)
                last = _emit_ctx(nc, pc, Vg, prev)
                nc.vector.tensor_copy(
                    ctxT[b][:, qb * QB:(qb + 1) * QB], pc[:]
                )
                return last

            def emit_a2a(b):
                cc_in = dram.tile([1024, 256], BF, name=f"cc_in{b}")
                cc_out = dram.tile([1024, 256], BF, name=f"cc_out{b}")
                nc.sync.dma_start(
                    cc_in[:].rearrange("(j p) c -> p j c", p=128),
                    ctxT[b][:].rearrange("p (j c) -> p j c", j=8),
                )
                nc.gpsimd.collective_compute(
                    "AllToAll",
                    mybir.AluOpType.bypass,
                    replica_groups=[list(range(NC))],
                    ins=[cc_in[:].opt()],
                    outs=[cc_out[:].opt()],
                )
                g = res.tile([128, 2048], BF, tag=f"cg{b}", name=f"cg{b}")
                nc.sync.dma_start(
                    g[:].rearrange("p (k c) -> p k c", k=8),
                    cc_out[:].rearrange("(k p) c -> p k c", p=128),
                )
                ctxg[b] = g

            # quarter-granular projections interleaved with attention:
            # b0-qb blocks need dT/Vg[0..3]; b1 needs [4..7].  D parts are
            # emitted ahead of V parts so the exp stream isn't starved.
            emit_proj_d(0)
            emit_proj_v(0)
            emit_proj_d(1)
            emit_proj_d(2)
            emit_proj_d(3)
            emit_proj_v(1)
            emit_proj_v(2)
            emit_proj_v(3)
            emit_attn_qblock(0, 0)
            emit_proj_d(4)
            emit_proj_v(4)
            emit_attn_qblock(0, 1)
            emit_proj_d(5)
            emit_proj_v(5)
            emit_attn_qblock(0, 2)
            emit_proj_d(6)
            emit_proj_v(6)
            emit_attn_qblock(0, 3)
            emit_proj_d(7)
            emit_proj_v(7)
            # deferred Wo load drains in DMA-queue slack from here on
            nc.sync.dma_start(
                wo_t[:].rearrange("p (k m) -> p k m", k=8),
                Wob[:].rearrange("(k p) m -> p k m", p=128),
            )
            emit_a2a(0)

            last_b1 = None
            for qb in range(NQB):
                last_b1 = emit_attn_qblock(1, qb)
            emit_a2a(1)

            # batch-0 output projection overlaps A2A(b1); the explicit dep
            # keeps it from preempting b1 attention at the PE queue head.
            first = emit_outproj_group(0, 0)
            add_dep_helper(
                _ins(first), _ins(last_b1), sync=False,
                reason="outproj(b0) after b1 attention",
            )
            for fo in range(1, 8):
                emit_outproj_group(0, fo)
            for fo in range(8):
                emit_outproj_group(1, fo)

    nc.compile()
    return nc


def _emit_ctx(nc, pc, Vg, prev):
    at, n, g, m, first, last = prev
    mm = None
    for h in range(2):
        mm = nc.tensor.matmul(
            pc[h * 64:(h + 1) * 64, 0:n],
            Vg[g][:, m * 128 + h * 64:m * 128 + h * 64 + 64],
            at[:, 512 * h:512 * h + n],
            start=first, stop=last,
            skip_group_check=True,
        )
    return mm


def make_in_maps(inputs):
    x = np.asarray(inputs["x"], np.float32)
    Wq = np.asarray(inputs["Wq"], np.float32)
    Wk = np.asarray(inputs["Wk"], np.float32)
    Wv = np.asarray(inputs["Wv"], np.float32)
    Wo = np.asarray(inputs["Wo"], np.float32)
    bo = np.asarray(inputs["bo"], np.float32)
    # bq/bk/bv are zeros by the problem's input spec; dif = x @ (Wq - Wk)
    # and v = x @ Wv absorb them exactly when zero.

    xT = np.ascontiguousarray(x.reshape(T, H).T).astype(np.float16)
    Wd = (Wq - Wk).astype(np.float16)
    Wob = Wo.astype(ml_dtypes.bfloat16)
    maskbf = np.tril(np.ones((128, 128), np.float32), -1).astype(ml_dtypes.bfloat16)

    in_maps = []
    for c in range(NC):
        cols = slice(c * 128, (c + 1) * 128)
        in_maps.append({
            "xT": xT,
            "Wdc": np.ascontiguousarray(Wd[:, cols]),
            "Wvc": np.ascontiguousarray(Wv[:, cols]).astype(np.float16),
            "Wob": Wob,
            "bo": bo,
            "maskbf": maskbf,
        })
    return in_maps


def kernel(**inputs):
    if "nc" not in _cached:
        _cached["nc"] = _build()
    nc = _cached["nc"]

    res = run_bass_kernel_spmd(nc, make_in_maps(inputs), core_ids=list(range(NC)))

    out = np.empty((B, S, H), np.float32)
    for c in range(NC):
        oT = res.results[c]["out"]                    # [H, 512]
        for b in range(B):
            out[b, c * 256:(c + 1) * 256, :] = oT[:, b * 256:(b + 1) * 256].T
    return out
